# revision 1
# baseline (speedup 1.0000x reference)
"""DirSageConv Trainium2 kernel (8 NeuronCores, SPMD).

Strategy (sharding_hint): nodes sharded across 8 cores; edges partitioned by
destination (resp. source) node so segment-mean scatter is local; every core
holds a full copy of x for the gather; small linears replicated.

Device algorithm per core / direction:
  - dma_gather pulls x[src] rows (256B each) from HBM.
  - dma_scatter_add accumulates them into a striped accumulator. The HW
    scatter-add races on duplicate indices, so edges are grouped by
    within-destination rank: a group = 2 consecutive ranks, stripe = rank%2
    (acc row = key_local + 16384*stripe) -> every scatter instruction sees
    unique rows. Tile's WAW ordering serializes scatter instructions.
  - epilogue: merge stripes, divide by degree, cast bf16, store, XBAR
    transpose-load -> matmul (W_in|b via ones-row trick) -> exact ELU
    (elu(z) = min(exp(z)-1, relu(z))) -> transposed output (host transposes).
  - self branch: xT via XBAR from a padded bf16 copy of x, two matmuls with
    ELU between, transposed output.
"""
import sys

sys.path.insert(0, "/opt/trn_rl_repo")

import ml_dtypes
import numpy as np

import concourse.bacc as bacc
import concourse.bass as bass
import concourse.mybir as mybir
from concourse import tile
from concourse.bass_utils import run_bass_kernel_spmd

F32 = mybir.dt.float32
BF16 = mybir.dt.bfloat16
I16 = mybir.dt.int16

AF = mybir.ActivationFunctionType
ALU = mybir.AluOpType


class P:
    """Problem/layout parameters (full scale defaults)."""

    def __init__(self, N=100000, F_IN=64, F_OUT=128, F_HID=512, NCORES=8,
                 BLOCK=32768, SS=16384):
        self.N, self.F_IN, self.F_OUT, self.F_HID = N, F_IN, F_OUT, F_HID
        self.NCORES = NCORES
        self.NPC = N // NCORES                       # real nodes per core
        self.NPAD = -(-self.NPC // 128) * 128 + 128  # acc rows incl pad tile
        self.SS = SS                                 # stripe stride in acc
        assert 2 * SS <= 32768 and self.NPAD < SS
        self.TRASH0 = self.NPAD
        self.TRASH_N = SS - self.NPAD
        self.SLICE_MAX = min(7424, (2 * self.TRASH_N) // 128 * 128)
        self.BLOCK = BLOCK
        self.NBLK = -(-N // BLOCK)
        self.NTILE = self.NPAD // 128                # epilogue tiles
        self.SELF_CH = -(-(self.NPAD - 128) // 512)  # 512-row self chunks
        self.XROWS = -(-(self.NPC * (NCORES - 1) + 512 * self.SELF_CH) // 128) * 128
        self.XROWS = max(self.XROWS, N)
        self.XROWS = -(-self.XROWS // 128) * 128


def _round_up(x, m):
    return (x + m - 1) // m * m


def _wrap_idx16(slots):
    """[C, TOT] int16 -> [C, 128, TOT//16] wrapped+replicated idx layout."""
    C, TOT = slots.shape
    a = slots.reshape(C, TOT // 16, 16).transpose(0, 2, 1)  # [C,16,TOT/16]
    return np.tile(a, (1, 8, 1)).copy()


def prep_direction(key, val, p: P):
    """Host-side edge partitioning for one direction.

    Returns (gidx [C,128,cols], sidx [C,128,cols], cnt [C,128,NTILE],
             slices) where slices = list of dicts:
      cols (c0,c1), n, subs=[(blk, a0, a1)]  -- a* are 128-row tile indices.
    """
    E = key.shape[0]
    core = key // p.NPC
    np.minimum(core, p.NCORES - 1, out=core)  # safety for key==N-1 edge cases
    kl = key - core * p.NPC

    order = np.argsort(key, kind="stable")
    ks = key[order]
    run_start_mask = np.empty(E, np.bool_)
    run_start_mask[0] = True
    np.not_equal(ks[1:], ks[:-1], out=run_start_mask[1:])
    run_id = np.cumsum(run_start_mask) - 1
    starts = np.flatnonzero(run_start_mask)
    rank_sorted = np.arange(E) - starts[run_id]
    rank = np.empty(E, np.int64)
    rank[order] = rank_sorted

    g = rank // 2
    stripe = rank % 2
    blk = val // p.BLOCK
    NG = int(g.max()) + 1
    NB = p.NBLK

    seg = (core * NG + g) * NB + blk
    cnt3 = np.bincount(seg, minlength=p.NCORES * NG * NB).reshape(
        p.NCORES, NG, NB)
    Pgb = cnt3.max(axis=0)  # [NG, NB]
    Pgb = np.where(Pgb > 0, ((Pgb + 127) // 128) * 128, 0)

    # layout: group-major, block-minor
    gtot = Pgb.sum(axis=1)  # [NG]
    goff = np.concatenate([[0], np.cumsum(gtot)])
    boff = np.zeros((NG, NB), np.int64)
    boff[:, 1:] = np.cumsum(Pgb[:, :-1], axis=1)
    TOT = int(goff[-1])
    TOT = _round_up(max(TOT, 128), 128)

    # slot index per edge: sort by (core, g, blk) stable
    order2 = np.argsort(seg, kind="stable")
    seg_sorted = seg[order2]
    m2 = np.empty(E, np.bool_)
    m2[0] = True
    np.not_equal(seg_sorted[1:], seg_sorted[:-1], out=m2[1:])
    sstarts = np.flatnonzero(m2)
    sid = np.cumsum(m2) - 1
    within = np.arange(E) - sstarts[sid]
    pos = np.empty(E, np.int64)
    pos[order2] = within
    slot = goff[g] + boff[g, blk] + pos

    # idx arrays
    j = np.arange(TOT)
    trash = (p.TRASH0 + (j % 2) * p.SS + (j // 2) % p.TRASH_N).astype(np.int16)
    gidx = np.zeros((p.NCORES, TOT), np.int16)
    sidx = np.tile(trash, (p.NCORES, 1))
    gval = (val - blk * p.BLOCK).astype(np.int16)
    sval = (kl + stripe * p.SS).astype(np.int16)
    gidx[core, slot] = gval
    sidx[core, slot] = sval

    # slices: cut each group's [goff[g], goff[g]+gtot[g]) into <=SLICE_MAX
    slices = []
    for gi in range(NG):
        g0, g1 = int(goff[gi]), int(goff[gi] + gtot[gi])
        if g1 == g0:
            continue
        npiece = -(-(g1 - g0) // p.SLICE_MAX)
        base = _round_up(-(-(g1 - g0) // npiece), 128)
        cuts = [g0 + min(base * k, g1 - g0) for k in range(npiece)] + [g1]
        for s0, s1 in zip(cuts[:-1], cuts[1:]):
            assert 0 < s1 - s0 <= p.SLICE_MAX
            subs = []
            for b in range(NB):
                b0 = int(goff[gi] + boff[gi, b])
                b1 = b0 + int(Pgb[gi, b])
                lo, hi = max(s0, b0), min(s1, b1)
                if hi > lo:
                    subs.append((b, (lo - s0) // 128, (hi - s0) // 128))
            slices.append(dict(cols=(s0 // 16, s1 // 16), n=s1 - s0,
                               s0=s0, subs=subs))

    # per-core degree, wrapped [128, NTILE]
    deg = np.bincount(key, minlength=p.NCORES * p.NPC).astype(np.float32)
    cnt = np.zeros((p.NCORES, 128, p.NTILE), np.float32)
    for c in range(p.NCORES):
        d = np.zeros(p.NPAD, np.float32)
        d[:p.NPC] = deg[c * p.NPC:(c + 1) * p.NPC]
        cnt[c] = d.reshape(p.NTILE, 128).T
    return _wrap_idx16(gidx), _wrap_idx16(sidx), cnt, slices


def _emit_elu(nc, pool, psum_ap, out_tile, n, out_dtype):
    """out = elu(psum) = min(exp(z)-1, relu(z)); psum [128, n]."""
    e = pool.tile([128, n], out_dtype, tag="elu_e")
    nc.scalar.activation(e[:, :n], psum_ap, AF.Exp)
    r = pool.tile([128, n], out_dtype, tag="elu_r")
    nc.vector.tensor_scalar_max(r[:, :n], psum_ap, 0.0)
    nc.vector.scalar_tensor_tensor(out_tile, e[:, :n], 1.0, r[:, :n],
                                   ALU.subtract, ALU.min)


def build_nc(p: P, slices_in, slices_out, idx_cols_in, idx_cols_out):
    nc = bacc.Bacc("TRN2", target_bir_lowering=False, debug=False,
                   enable_asserts=True, dynamic_dma_scratch_size=32768)
    FI, FO, FH = p.F_IN, p.F_OUT, p.F_HID

    x_d = nc.dram_tensor("x", [p.XROWS, FI], F32, kind="ExternalInput")
    xs_d = nc.dram_tensor("xs", [512 * p.SELF_CH, FO], BF16,
                          kind="ExternalInput")  # per-core bf16 x slice, padded
    wbin_d = nc.dram_tensor("wb_in", [FI + 1, FO], BF16, kind="ExternalInput")
    wbout_d = nc.dram_tensor("wb_out", [FI + 1, FO], BF16, kind="ExternalInput")
    wb1_d = nc.dram_tensor("wb1", [FI + 1, FH], BF16, kind="ExternalInput")
    w2p_d = nc.dram_tensor("w2p", [128, FH // 128 * FO], BF16,
                           kind="ExternalInput")
    b2_d = nc.dram_tensor("b2", [1, FO], BF16, kind="ExternalInput")
    dirs = []
    for name, slices, cols in (("in", slices_in, idx_cols_in),
                               ("out", slices_out, idx_cols_out)):
        d = dict(
            name=name, slices=slices,
            gidx=nc.dram_tensor(f"gidx_{name}", [128, cols], I16,
                                kind="ExternalInput"),
            sidx=nc.dram_tensor(f"sidx_{name}", [128, cols], I16,
                                kind="ExternalInput"),
            cnt=nc.dram_tensor(f"cnt_{name}", [128, p.NTILE], F32,
                               kind="ExternalInput"),
            acc=nc.dram_tensor(f"acc_{name}", [2 * p.SS, FI], F32,
                               kind="Internal"),
            mean=nc.dram_tensor(f"mean_{name}", [p.NPAD, 128], BF16,
                                kind="Internal"),
            yT=nc.dram_tensor(f"yT_{name}", [128, p.NPAD], F32,
                              kind="ExternalOutput"),
            wb=wbin_d if name == "in" else wbout_d,
        )
        dirs.append(d)
    yself_d = nc.dram_tensor("yT_self", [128, 512 * p.SELF_CH], F32,
                             kind="ExternalOutput")

    with tile.TileContext(nc) as tc:
        with tc.tile_pool(name="const", bufs=1) as cpool, \
             tc.tile_pool(name="idx", bufs=4) as ipool, \
             tc.tile_pool(name="feat", bufs=4) as fpool, \
             tc.tile_pool(name="ep", bufs=3) as epool, \
             tc.tile_pool(name="meanT", bufs=1) as mpool, \
             tc.tile_pool(name="selfp", bufs=3) as spool, \
             tc.tile_pool(name="ps", bufs=2, space="PSUM") as pspool, \
             tc.tile_pool(name="ps2", bufs=2, space="PSUM") as ps2pool:

            zero_sb = cpool.tile([128, 1024], F32)
            nc.vector.memset(zero_sb[:], 0.0)

            # ---- scatter/gather phase per direction ----
            for d in dirs:
                acc = d["acc"]
                # zero-init real rows of both stripes
                for s in range(2):
                    r = 0
                    while r < p.NPAD:
                        nrows = min(16, (p.NPAD - r) // 128)
                        out = acc[s * p.SS + r:s * p.SS + r + nrows * 128, :]
                        out = out.rearrange("(n q) f -> q n f", q=128)
                        nc.sync.dma_start(
                            out, zero_sb[:, :nrows * 64].rearrange(
                                "q (n f) -> q n f", f=FI))
                        r += nrows * 128
                for sl in d["slices"]:
                    c0, c1 = sl["cols"]
                    n = sl["n"]
                    gi = ipool.tile([128, c1 - c0], I16, tag="gi")
                    nc.sync.dma_start(gi[:], d["gidx"][:, c0:c1])
                    si = ipool.tile([128, c1 - c0], I16, tag="si")
                    nc.sync.dma_start(si[:], d["sidx"][:, c0:c1])
                    feat = fpool.tile([128, p.SLICE_MAX // 128, FI], F32,
                                      tag="feat")
                    for (b, a0, a1) in sl["subs"]:
                        rows = min(p.BLOCK, p.XROWS - b * p.BLOCK)
                        xb = x_d[b * p.BLOCK:b * p.BLOCK + rows, :]
                        nsub = (a1 - a0) * 128
                        nc.gpsimd.dma_gather(
                            feat[:, a0:a1, :], xb,
                            gi[:, (a0 * 8):(a1 * 8)], nsub, nsub, FI,
                            single_packet=False)
                    nc.gpsimd.dma_scatter_add(
                        acc[:], feat[:, :n // 128, :], si[:, :n // 16],
                        n, n, FI, single_packet=False)

            # ---- epilogue per direction ----
            for d in dirs:
                acc, mean = d["acc"], d["mean"]
                cntt = epool.tile([128, p.NTILE], F32, tag="cnt")
                nc.sync.dma_start(cntt[:], d["cnt"][:])
                cntm = epool.tile([128, p.NTILE], F32, tag="cntm")
                nc.vector.tensor_scalar_max(cntm[:], cntt[:], 1.0)
                recip = epool.tile([128, p.NTILE], F32, tag="recip")
                nc.vector.reciprocal(recip[:], cntm[:])

                for t in range(p.NTILE):
                    s01 = epool.tile([128, 2, FI], F32, tag="s01")
                    src = acc[:].rearrange("(s r) f -> s r f", s=2)[
                        :, 128 * t:128 * (t + 1), :].rearrange(
                        "s q f -> q s f")
                    nc.sync.dma_start(s01[:], src)
                    summ = epool.tile([128, FI], F32, tag="summ")
                    nc.vector.tensor_add(summ[:], s01[:, 0, :], s01[:, 1, :])
                    mt = epool.tile([128, 128], BF16, tag="mt")
                    nc.vector.tensor_scalar_mul(mt[:, 0:FI], summ[:],
                                                recip[:, t:t + 1])
                    nc.vector.memset(mt[:, FI:128], 1.0)
                    nc.sync.dma_start(mean[128 * t:128 * (t + 1), :], mt[:])

                meanT = mpool.tile([128, p.NPAD], BF16, tag="meanT")
                nc.sync.dma_start_transpose(meanT[:], mean[:])
                wb = epool.tile([FI + 1, FO], BF16, tag="wb")
                nc.sync.dma_start(wb[:], d["wb"][:])
                n0 = 0
                while n0 < p.NPAD:
                    n = min(512, p.NPAD - n0)
                    ps = pspool.tile([128, 512], F32, tag="dpsum")
                    nc.tensor.matmul(ps[:, :n], wb[:], meanT[0:FI + 1,
                                                             n0:n0 + n],
                                     start=True, stop=True)
                    yt = epool.tile([128, 512], F32, tag="yt")
                    _emit_elu(nc, epool, ps[:, :n], yt[:, :n], n, F32)
                    nc.sync.dma_start(d["yT"][:, n0:n0 + n], yt[:, :n])
                    n0 += n

            # ---- self branch ----
            wb1 = cpool.tile([FI + 1, FH], BF16)
            nc.sync.dma_start(wb1[:], wb1_d[:])
            w2p = cpool.tile([128, FH // 128 * FO], BF16)
            nc.sync.dma_start(w2p[:], w2p_d[:])
            b2t = cpool.tile([1, FO], BF16)
            nc.sync.dma_start(b2t[:], b2_d[:])
            ones_row = cpool.tile([1, 512], BF16)
            nc.vector.memset(ones_row[:], 1.0)
            nk = FH // 128
            for t in range(p.SELF_CH):
                xT = spool.tile([128, 512], BF16, tag="xT")
                nc.scalar.dma_start_transpose(
                    xT[:], xs_d[512 * t:512 * (t + 1), :])
                ps2 = ps2pool.tile([128, 512], F32, tag="ps2")
                for k in range(nk):
                    ps1 = pspool.tile([128, 512], F32, tag="ps1")
                    nc.tensor.matmul(ps1[:], wb1[:, 128 * k:128 * (k + 1)],
                                     xT[0:FI + 1, :], start=True, stop=True)
                    hk = spool.tile([128, 512], BF16, tag="hk")
                    _emit_elu(nc, spool, ps1[:], hk[:], 512, BF16)
                    nc.tensor.matmul(ps2[:], w2p[:, FO * k:FO * (k + 1)],
                                     hk[:], start=(k == 0), stop=False)
                nc.tensor.matmul(ps2[:], b2t[:], ones_row[:],
                                 start=False, stop=True)
                yt = spool.tile([128, 512], F32, tag="yts")
                _emit_elu(nc, spool, ps2[:], yt[:], 512, F32)
                nc.sync.dma_start(yself_d[:, 512 * t:512 * (t + 1)], yt[:])

    nc.compile()
    return nc


def run(inputs, p: P, trace=False):
    x = np.asarray(inputs["x"], np.float32)
    ei = np.asarray(inputs["edge_index"], np.int64)
    src, dst = ei[0], ei[1]

    gin, sin, cin, slices_in = prep_direction(dst, src, p)
    gout, sout, cout, slices_out = prep_direction(src, dst, p)

    xdev = np.zeros((p.XROWS, p.F_IN), np.float32)
    xdev[:p.N] = x
    # bf16 padded x with ones marker col, per-core slices
    xb = np.zeros((p.XROWS, 128), np.float32)
    xb[:p.N, :p.F_IN] = x
    xb[:, p.F_IN] = 1.0
    xb16 = xb.astype(ml_dtypes.bfloat16)

    def bf(a):
        return np.asarray(a, np.float32).astype(ml_dtypes.bfloat16)

    wbin = np.vstack([inputs["W_in"], np.asarray(inputs["b_in"])[None, :]])
    wbout = np.vstack([inputs["W_out"], np.asarray(inputs["b_out"])[None, :]])
    wb1 = np.vstack([inputs["W1"], np.asarray(inputs["b1"])[None, :]])
    W2 = np.asarray(inputs["W2"], np.float32)
    w2p = np.zeros((128, (p.F_HID // 128) * p.F_OUT), np.float32)
    for k in range(p.F_HID // 128):
        w2p[:, k * p.F_OUT:(k + 1) * p.F_OUT] = W2[k * 128:(k + 1) * 128, :]
    b2 = np.asarray(inputs["b2"], np.float32)[None, :]

    nc = build_nc(p, slices_in, slices_out, gin.shape[2], gout.shape[2])

    in_maps = []
    for c in range(p.NCORES):
        r0 = c * p.NPC
        xs = np.zeros((512 * p.SELF_CH, 128), ml_dtypes.bfloat16)
        take = min(512 * p.SELF_CH, p.XROWS - r0)
        xs[:take] = xb16[r0:r0 + take]
        in_maps.append({
            "x": xdev, "xs": xs,
            "wb_in": bf(wbin), "wb_out": bf(wbout), "wb1": bf(wb1),
            "w2p": bf(w2p), "b2": bf(b2),
            "gidx_in": gin[c], "sidx_in": sin[c], "cnt_in": cin[c],
            "gidx_out": gout[c], "sidx_out": sout[c], "cnt_out": cout[c],
        })

    kw = {}
    if trace:
        kw = dict(trace=True, trace_cores=[0])
    res = run_bass_kernel_spmd(nc, in_maps, core_ids=list(range(p.NCORES)),
                               **kw)

    def gather_out(name):
        return np.concatenate(
            [res.results[c][name][:, :p.NPC].T for c in range(p.NCORES)], 0)

    x_in = gather_out("yT_in")
    x_out = gather_out("yT_out")
    x_self = gather_out("yT_self")
    return (x_in, x_out, x_self), res


def kernel(**inputs):
    p = P()
    (x_in, x_out, x_self), _ = run(inputs, p, trace=False)
    return x_in, x_out, x_self



# revision 8
# speedup vs baseline: 1.2652x; 1.2652x over previous
"""DirSageConv Trainium2 kernel (8 NeuronCores, SPMD) — one-hot matmul design.

Strategy: nodes sharded across 8 cores; edges partitioned by destination
(resp. source) so the segment-mean lands in core-local PSUM windows; the
small linears are replicated.

Per direction (in: aggregate x[src] at dst; out: aggregate x[dst] at src):
  - Edges are grouped on the host by (core, src-pair block, dst-window) and
    padded to 128-edge tiles.  dst-window = 128 consecutive core-local keys.
  - dma_gather fetches the PAIR row (512B, two consecutive x rows, f32) of
    each edge's source node into SBUF in edge-slot order.  Pair indices fit
    int16 in two blocks (p < 32768, p >= 32768) -> two passes.
  - Scalar engine casts gathered chunks to bf16.
  - For each 128-edge tile, DVE builds a [128, 256] one-hot T via
    tensor_scalar(is_equal, dlc) * rec where dlc = dstlocal + 128*h
    (h = which row of the pair is the real source) and rec = 1/deg(dst).
    One matmul per tile accumulates  psum[pf, d0:128 | d1:128] += featb^T T.
  - Window epilogue: mean^T = psum[0:64, 0:128] + psum[64:128, 128:256]
    (pass A stashes this partial in SBUF; pass B adds it back), append a
    ones row, matmul with [W|b], exact ELU, DMA out (output transposed;
    host transposes back).
  - Self branch: two matmuls with ELU via xbar-transposed loads of x.
"""
import sys

sys.path.insert(0, "/opt/trn_rl_repo")

import ml_dtypes
import numpy as np

import concourse.bacc as bacc
import concourse.mybir as mybir
from concourse import tile
from concourse.bass_utils import run_bass_kernel_spmd

F32 = mybir.dt.float32
BF16 = mybir.dt.bfloat16
I16 = mybir.dt.int16

AF = mybir.ActivationFunctionType
ALU = mybir.AluOpType

N = 100000
F_IN = 64
F_OUT = 128
F_HID = 512
NCORES = 8
NPC = N // NCORES              # 12500 nodes per core
NWIN = -(-NPC // 128)          # 98 windows of 128 dsts
NPAD = NWIN * 128              # 12544
NPAIR_ROWS = 100096            # x padded rows (pairs: 50048)
PBLK = 32768                   # pair-index block size (int16 reach)
PAD_DLC = 300.0                # one-hot miss marker (> 255)
CH_TILES = 32                  # tiles per dma_gather chunk
SELF_CH = 25                   # 512-row self-branch chunks (12800 rows)
SP = False                     # single_packet=True wedges the device; keep False


def _wrap_idx16(slots):
    """[C, TOT] int16 -> [C, 128, TOT//16] wrapped+replicated idx layout."""
    C, TOT = slots.shape
    a = slots.reshape(C, TOT // 16, 16).transpose(0, 2, 1)
    return np.tile(a, (1, 8, 1)).copy()


def prep_direction(key, val, deg):
    """Host edge partitioning for one direction.

    key: aggregation node (dst for 'in'), val: gathered node (src for 'in').
    Returns (gidx [C,128,cols], dlc [C,128,NT], rec [C,128,NT],
             wtiles list[(blk, w, ntiles)] shared across cores, NT).
    """
    E = key.shape[0]
    core = np.minimum(key // NPC, NCORES - 1)
    kl = key - core * NPC
    w = kl >> 7
    d = kl & 127
    p = val >> 1
    h = val & 1
    blk = (p >= PBLK).astype(np.int64)

    # group id: (core, blk, w)
    gid = (core * 2 + blk) * NWIN + w
    NG = NCORES * 2 * NWIN
    cnt = np.bincount(gid, minlength=NG).reshape(NCORES, 2 * NWIN)
    tiles_bw = -(-cnt // 128)            # [C, 2*NWIN]
    tiles_shared = np.maximum(tiles_bw.max(axis=0), 1)   # [2*NWIN]
    NT = int(tiles_shared.sum())
    base = np.concatenate([[0], np.cumsum(tiles_shared)])[:-1]  # tile base

    order = np.argsort(gid, kind="stable")
    gs = gid[order]
    m = np.empty(E, np.bool_)
    m[0] = True
    np.not_equal(gs[1:], gs[:-1], out=m[1:])
    starts = np.flatnonzero(m)
    sid = np.cumsum(m) - 1
    within = np.arange(E) - starts[sid]
    pos = np.empty(E, np.int64)
    pos[order] = within
    bw = gid % (2 * NWIN)
    slot = base[bw] * 128 + pos          # slot within the core's stream

    TOT = NT * 128
    gidx = np.ones((NCORES, TOT), np.int16)       # pad: pair 1 (valid row)
    dlc_a = np.full((NCORES, TOT), PAD_DLC, np.float32)
    rec_a = np.zeros((NCORES, TOT), np.float32)

    p16 = (p - blk * PBLK).astype(np.int16)
    gidx[core, slot] = p16
    dlc_a[core, slot] = (d + 128 * h).astype(np.float32)
    rec_a[core, slot] = 1.0 / np.maximum(deg[key], 1.0)

    dlc = dlc_a.reshape(NCORES, NT, 128).transpose(0, 2, 1)
    rec = rec_a.reshape(NCORES, NT, 128).transpose(0, 2, 1)
    dlc = np.ascontiguousarray(dlc)
    rec = np.ascontiguousarray(rec)

    wtiles = []
    for b in range(2):
        for wi in range(NWIN):
            wtiles.append((b, wi, int(tiles_shared[b * NWIN + wi])))
    return _wrap_idx16(gidx), dlc, rec, wtiles, NT


def _emit_elu(nc, pool, psum_ap, out_tile, n, out_dtype):
    """out = elu(psum) = min(exp(z)-1, relu(z))."""
    e = pool.tile([128, n], out_dtype, tag="elu_e")
    nc.scalar.activation(e[:, :n], psum_ap, AF.Exp)
    r = pool.tile([128, n], out_dtype, tag="elu_r")
    nc.vector.tensor_scalar_max(r[:, :n], psum_ap, 0.0)
    nc.vector.scalar_tensor_tensor(out_tile, e[:, :n], 1.0, r[:, :n],
                                   ALU.subtract, ALU.min)


def build_nc(wt_in, nt_in, wt_out, nt_out):
    nc = bacc.Bacc("TRN2", target_bir_lowering=False, debug=False,
                   enable_asserts=True, dynamic_dma_scratch_size=32768)

    x_d = nc.dram_tensor("x", [NPAIR_ROWS, F_IN], F32, kind="ExternalInput")
    xp = x_d.rearrange("(p two) f -> p (two f)", two=2)   # [50048, 128]
    xs_d = nc.dram_tensor("xs", [512 * SELF_CH, F_OUT], BF16,
                          kind="ExternalInput")
    iota_d = nc.dram_tensor("iota", [128, 256], BF16, kind="ExternalInput")
    wbin_d = nc.dram_tensor("wb_in", [F_IN + 1, F_OUT], BF16,
                            kind="ExternalInput")
    wbout_d = nc.dram_tensor("wb_out", [F_IN + 1, F_OUT], BF16,
                             kind="ExternalInput")
    wb1_d = nc.dram_tensor("wb1", [F_IN + 1, F_HID], BF16,
                           kind="ExternalInput")
    w2p_d = nc.dram_tensor("w2p", [128, F_HID // 128 * F_OUT], BF16,
                           kind="ExternalInput")
    b2_d = nc.dram_tensor("b2", [1, F_OUT], BF16, kind="ExternalInput")

    dirs = []
    for name, wt, ntt in (("in", wt_in, nt_in), ("out", wt_out, nt_out)):
        dirs.append(dict(
            name=name, wtiles=wt, NT=ntt,
            gidx=nc.dram_tensor(f"gidx_{name}", [128, ntt * 8], I16,
                                kind="ExternalInput"),
            dlc=nc.dram_tensor(f"dlc_{name}", [128, ntt], F32,
                               kind="ExternalInput"),
            rec=nc.dram_tensor(f"rec_{name}", [128, ntt], F32,
                               kind="ExternalInput"),
            yT=nc.dram_tensor(f"yT_{name}", [128, NPAD], F32,
                              kind="ExternalOutput"),
            wb=wbin_d if name == "in" else wbout_d,
        ))
    yself_d = nc.dram_tensor("yT_self", [128, 512 * SELF_CH], F32,
                             kind="ExternalOutput")

    with tile.TileContext(nc) as tc:
        with tc.tile_pool(name="const", bufs=1) as cpool, \
             tc.tile_pool(name="meta", bufs=2) as mpool, \
             tc.tile_pool(name="idx", bufs=4) as ipool, \
             tc.tile_pool(name="feat", bufs=2) as fpool, \
             tc.tile_pool(name="featb", bufs=2) as fbpool, \
             tc.tile_pool(name="T", bufs=6) as tpool, \
             tc.tile_pool(name="ep", bufs=4) as epool, \
             tc.tile_pool(name="hacc", bufs=1) as hpool, \
             tc.tile_pool(name="selfp", bufs=3) as spool, \
             tc.tile_pool(name="psAB", bufs=2, space="PSUM") as pspool, \
             tc.tile_pool(name="psY", bufs=2, space="PSUM") as pypool, \
             tc.tile_pool(name="ps2", bufs=2, space="PSUM") as ps2pool:

            iota_t = cpool.tile([128, 256], BF16)
            nc.sync.dma_start(iota_t[:], iota_d[:])

            for d in dirs:
                NT = d["NT"]
                dlc_t = mpool.tile([128, NT], F32, tag="dlc")
                nc.sync.dma_start(dlc_t[:], d["dlc"][:])
                rec_t = mpool.tile([128, NT], F32, tag="rec")
                nc.sync.dma_start(rec_t[:], d["rec"][:])
                wb_t = mpool.tile([F_IN + 1, F_OUT], BF16, tag="wb")
                nc.sync.dma_start(wb_t[:], d["wb"][:])
                hacc = hpool.tile([F_IN, NPAD], F32, tag="hacc")

                # chunk boundaries: consecutive tiles within one pass
                chunks = {}   # gt -> (chunk tiles, chunk first gt)
                gt = 0
                for b in range(2):
                    p0 = gt
                    ntp = sum(n for bb, _, n in d["wtiles"] if bb == b)
                    while gt < p0 + ntp:
                        n = min(CH_TILES, p0 + ntp - gt)
                        chunks[gt] = n
                        gt += n

                gt = 0
                feat_t = None
                featb_t = None
                ch0 = 0
                for (b, wi, ntw) in d["wtiles"]:
                    for k in range(ntw):
                        if gt in chunks:
                            ncht = chunks[gt]
                            ch0 = gt
                            gi = ipool.tile([128, ncht * 8], I16, tag="gi")
                            nc.sync.dma_start(
                                gi[:], d["gidx"][:, gt * 8:(gt + ncht) * 8])
                            feat_t = fpool.tile([128, CH_TILES, 128], F32,
                                                tag="feat")
                            src = xp[0:PBLK, :] if b == 0 else \
                                xp[PBLK:50048, :]
                            nidx = ncht * 128
                            nc.gpsimd.dma_gather(
                                feat_t[:, :ncht, :], src, gi[:], nidx, nidx,
                                128, single_packet=SP)
                            featb_t = fbpool.tile([128, CH_TILES, 128], BF16,
                                                  tag="featb")
                            nc.scalar.activation(
                                featb_t[:, :ncht, :].rearrange(
                                    "p n f -> p (n f)"),
                                feat_t[:, :ncht, :].rearrange(
                                    "p n f -> p (n f)"),
                                AF.Copy)
                        o = gt - ch0
                        Tt = tpool.tile([128, 256], BF16, tag="T")
                        nc.vector.tensor_scalar(
                            Tt[:], iota_t[:], dlc_t[:, gt:gt + 1],
                            rec_t[:, gt:gt + 1], ALU.is_equal, ALU.mult)
                        if k == 0:
                            psAB = pspool.tile([F_IN, 128], F32, tag="psAB")
                        nc.tensor.matmul(psAB[:],
                                         featb_t[:, o, 0:F_IN], Tt[:, 0:128],
                                         start=(k == 0), stop=False)
                        nc.tensor.matmul(psAB[:],
                                         featb_t[:, o, F_IN:128],
                                         Tt[:, 128:256],
                                         start=False, stop=(k == ntw - 1))
                        gt += 1

                    # window epilogue
                    wsl = slice(wi * 128, (wi + 1) * 128)
                    if b == 0:
                        nc.scalar.activation(hacc[:, wsl], psAB[:], AF.Copy)
                    else:
                        hT = epool.tile([F_IN + 1, 128], BF16, tag="hT")
                        nc.vector.tensor_tensor(
                            hT[0:F_IN, :], psAB[:], hacc[:, wsl], ALU.add)
                        nc.vector.memset(hT[F_IN:F_IN + 1, :], 1.0)
                        psY = pypool.tile([128, 128], F32, tag="psY")
                        nc.tensor.matmul(psY[:], wb_t[:], hT[:],
                                         start=True, stop=True)
                        y = epool.tile([128, 128], F32, tag="y")
                        _emit_elu(nc, epool, psY[:], y[:], 128, F32)
                        nc.sync.dma_start(d["yT"][:, wsl], y[:])

            # ---- self branch ----
            wb1 = cpool.tile([F_IN + 1, F_HID], BF16)
            nc.sync.dma_start(wb1[:], wb1_d[:])
            w2p = cpool.tile([128, F_HID // 128 * F_OUT], BF16)
            nc.sync.dma_start(w2p[:], w2p_d[:])
            b2t = cpool.tile([1, F_OUT], BF16)
            nc.sync.dma_start(b2t[:], b2_d[:])
            ones_row = cpool.tile([1, 512], BF16)
            nc.vector.memset(ones_row[:], 1.0)
            nk = F_HID // 128
            for t in range(SELF_CH):
                xT = spool.tile([128, 512], BF16, tag="xT")
                nc.scalar.dma_start_transpose(
                    xT[:], xs_d[512 * t:512 * (t + 1), :])
                ps2 = ps2pool.tile([128, 512], F32, tag="ps2")
                for k in range(nk):
                    ps1 = pypool.tile([128, 512], F32, tag="ps1")
                    nc.tensor.matmul(ps1[:], wb1[:, 128 * k:128 * (k + 1)],
                                     xT[0:F_IN + 1, :], start=True, stop=True)
                    hk = spool.tile([128, 512], BF16, tag="hk")
                    _emit_elu(nc, spool, ps1[:], hk[:], 512, BF16)
                    nc.tensor.matmul(ps2[:], w2p[:, F_OUT * k:F_OUT * (k + 1)],
                                     hk[:], start=(k == 0), stop=False)
                nc.tensor.matmul(ps2[:], b2t[:], ones_row[:],
                                 start=False, stop=True)
                yt = spool.tile([128, 512], F32, tag="yts")
                _emit_elu(nc, spool, ps2[:], yt[:], 512, F32)
                nc.sync.dma_start(yself_d[:, 512 * t:512 * (t + 1)], yt[:])

    nc.compile()
    return nc


def run(inputs, trace=False):
    x = np.asarray(inputs["x"], np.float32)
    ei = np.asarray(inputs["edge_index"], np.int64)
    src, dst = ei[0], ei[1]

    deg_dst = np.bincount(dst, minlength=N).astype(np.float32)
    deg_src = np.bincount(src, minlength=N).astype(np.float32)

    gin, dlin, recin, wt_in, nt_in = prep_direction(dst, src, deg_dst)
    gout, dlout, recout, wt_out, nt_out = prep_direction(src, dst, deg_src)

    xdev = np.zeros((NPAIR_ROWS, F_IN), np.float32)
    xdev[:N] = x

    # bf16 padded x with ones marker col for the self branch
    xb = np.zeros((NPAIR_ROWS, 128), np.float32)
    xb[:N, :F_IN] = x
    xb[:, F_IN] = 1.0
    xb16 = xb.astype(ml_dtypes.bfloat16)

    def bf(a):
        return np.asarray(a, np.float32).astype(ml_dtypes.bfloat16)

    wbin = np.vstack([inputs["W_in"], np.asarray(inputs["b_in"])[None, :]])
    wbout = np.vstack([inputs["W_out"], np.asarray(inputs["b_out"])[None, :]])
    wb1 = np.vstack([inputs["W1"], np.asarray(inputs["b1"])[None, :]])
    W2 = np.asarray(inputs["W2"], np.float32)
    w2p = np.zeros((128, (F_HID // 128) * F_OUT), np.float32)
    for k in range(F_HID // 128):
        w2p[:, k * F_OUT:(k + 1) * F_OUT] = W2[k * 128:(k + 1) * 128, :]
    b2 = np.asarray(inputs["b2"], np.float32)[None, :]

    iota = np.tile(np.arange(256, dtype=np.float32)[None, :], (128, 1))
    iota16 = iota.astype(ml_dtypes.bfloat16)

    nc = build_nc(wt_in, nt_in, wt_out, nt_out)

    in_maps = []
    for c in range(NCORES):
        r0 = c * NPC
        xs = np.zeros((512 * SELF_CH, 128), ml_dtypes.bfloat16)
        take = min(512 * SELF_CH, NPAIR_ROWS - r0)
        xs[:take] = xb16[r0:r0 + take]
        in_maps.append({
            "x": xdev, "xs": xs, "iota": iota16,
            "wb_in": bf(wbin), "wb_out": bf(wbout), "wb1": bf(wb1),
            "w2p": bf(w2p), "b2": bf(b2),
            "gidx_in": gin[c], "dlc_in": dlin[c], "rec_in": recin[c],
            "gidx_out": gout[c], "dlc_out": dlout[c], "rec_out": recout[c],
        })

    kw = {}
    if trace:
        kw = dict(trace=True, trace_cores=[0])
    res = run_bass_kernel_spmd(nc, in_maps, core_ids=list(range(NCORES)),
                               **kw)

    def gather_out(name):
        return np.concatenate(
            [res.results[c][name][:, :NPC].T for c in range(NCORES)], 0)

    x_in = gather_out("yT_in")
    x_out = gather_out("yT_out")
    x_self = gather_out("yT_self")
    return (x_in, x_out, x_self), res


def kernel(**inputs):
    (x_in, x_out, x_self), _ = run(inputs, trace=False)
    return x_in, x_out, x_self


# revision 12
# speedup vs baseline: 1.5702x; 1.2411x over previous
"""DirSageConv Trainium2 kernel (8 NeuronCores, SPMD) — one-hot matmul design.

Strategy: nodes sharded across 8 cores; edges partitioned by destination
(resp. source) so the segment-mean lands in core-local PSUM windows; the
small linears are replicated.

Per direction (in: aggregate x[src] at dst; out: aggregate x[dst] at src):
  - Edges are grouped on the host by (core, src-pair block, dst-window) and
    padded to 128-edge tiles.  dst-window = 128 consecutive core-local keys.
  - dma_gather fetches the PAIR row (512B, two consecutive x rows, f32) of
    each edge's source node into SBUF in edge-slot order.  Pair indices fit
    int16 in two blocks (p < 32768, p >= 32768) -> two passes.
  - Scalar engine casts gathered chunks to bf16.
  - For each 128-edge tile, DVE builds a [128, 256] one-hot T via
    tensor_scalar(is_equal, dlc) * rec where dlc = dstlocal + 128*h
    (h = which row of the pair is the real source) and rec = 1/deg(dst).
    One matmul per tile accumulates  psum[pf, d0:128 | d1:128] += featb^T T.
  - Window epilogue: mean^T = psum[0:64, 0:128] + psum[64:128, 128:256]
    (pass A stashes this partial in SBUF; pass B adds it back), append a
    ones row, matmul with [W|b], exact ELU, DMA out (output transposed;
    host transposes back).
  - Self branch: two matmuls with ELU via xbar-transposed loads of x.
"""
import sys

sys.path.insert(0, "/opt/trn_rl_repo")

import ml_dtypes
import numpy as np

import concourse.bacc as bacc
import concourse.mybir as mybir
from concourse import tile
from concourse.bass_utils import run_bass_kernel_spmd

F32 = mybir.dt.float32
BF16 = mybir.dt.bfloat16
I16 = mybir.dt.int16
FP16 = mybir.dt.float16

AF = mybir.ActivationFunctionType
ALU = mybir.AluOpType

N = 100000
F_IN = 64
F_OUT = 128
F_HID = 512
NCORES = 8
NPC = N // NCORES              # 12500 nodes per core
WIN = 256                      # dsts per window
NWIN = -(-NPC // WIN)          # 49 windows of 256 dsts
NPAD = NWIN * WIN              # 12544
NPAIR_ROWS = 100096            # x padded rows (pairs: 50048)
PBLK = 32768                   # pair-index block size (int16 reach)
PAD_DLC = 600.0                # one-hot miss marker (> 511)
CH_TILES = 32                  # tiles per dma_gather chunk
SELF_CH = 25                   # 512-row self-branch chunks (12800 rows)
SP = False                     # single_packet=True wedges the device; keep False
TB = 4                         # tiles per one-hot build instruction


def _wrap_idx16(slots):
    """[C, TOT] int16 -> [C, 128, TOT//16] wrapped+replicated idx layout."""
    C, TOT = slots.shape
    a = slots.reshape(C, TOT // 16, 16).transpose(0, 2, 1)
    return np.tile(a, (1, 8, 1)).copy()


def prep_direction(key, val, deg):
    """Host edge partitioning for one direction.

    key: aggregation node (dst for 'in'), val: gathered node (src for 'in').
    Returns (gidx [C,128,cols], dlc [C,128,NT],
             wtiles list[(blk, w, ntiles)] shared across cores, NT).
    """
    E = key.shape[0]
    core = np.minimum(key // NPC, NCORES - 1)
    kl = key - core * NPC
    w = kl // WIN
    d = kl % WIN
    p = val >> 1
    h = val & 1
    blk = (p >= PBLK).astype(np.int64)

    # group id: (core, blk, w)
    gid = (core * 2 + blk) * NWIN + w
    NG = NCORES * 2 * NWIN
    cnt = np.bincount(gid, minlength=NG).reshape(NCORES, 2 * NWIN)
    tiles_bw = -(-cnt // 128)            # [C, 2*NWIN]
    tiles_shared = np.maximum(tiles_bw.max(axis=0), 1)   # [2*NWIN]
    NT = int(tiles_shared.sum())
    base = np.concatenate([[0], np.cumsum(tiles_shared)])[:-1]  # tile base

    order = np.argsort(gid, kind="stable")
    gs = gid[order]
    m = np.empty(E, np.bool_)
    m[0] = True
    np.not_equal(gs[1:], gs[:-1], out=m[1:])
    starts = np.flatnonzero(m)
    sid = np.cumsum(m) - 1
    within = np.arange(E) - starts[sid]
    pos = np.empty(E, np.int64)
    pos[order] = within
    bw = gid % (2 * NWIN)
    slot = base[bw] * 128 + pos          # slot within the core's stream

    TOT = NT * 128
    gidx = np.ones((NCORES, TOT), np.int16)       # pad: pair 1 (valid row)
    dlc_a = np.full((NCORES, TOT), PAD_DLC, np.float32)

    p16 = (p - blk * PBLK).astype(np.int16)
    gidx[core, slot] = p16
    dlc_a[core, slot] = (d + WIN * h).astype(np.float32)

    dlc = dlc_a.reshape(NCORES, NT, 128).transpose(0, 2, 1)
    dlc = np.ascontiguousarray(dlc).astype(np.float16)

    wtiles = []
    for b in range(2):
        for wi in range(NWIN):
            wtiles.append((b, wi, int(tiles_shared[b * NWIN + wi])))
    return _wrap_idx16(gidx), dlc, wtiles, NT


def _emit_elu(nc, pool, psum_ap, out_tile, n, out_dtype):
    """out = elu(psum) = min(exp(z)-1, relu(z))."""
    e = pool.tile([128, n], out_dtype, tag="elu_e")
    nc.scalar.activation(e[:, :n], psum_ap, AF.Exp)
    r = pool.tile([128, n], out_dtype, tag="elu_r")
    nc.vector.tensor_scalar_max(r[:, :n], psum_ap, 0.0)
    nc.vector.scalar_tensor_tensor(out_tile, e[:, :n], 1.0, r[:, :n],
                                   ALU.subtract, ALU.min)


def build_nc(wt_in, nt_in, wt_out, nt_out):
    nc = bacc.Bacc("TRN2", target_bir_lowering=False, debug=False,
                   enable_asserts=True, dynamic_dma_scratch_size=32768)

    x_d = nc.dram_tensor("x", [NPAIR_ROWS, F_IN], F32, kind="ExternalInput")
    xp = x_d.rearrange("(p two) f -> p (two f)", two=2)   # [50048, 128]
    xs_d = nc.dram_tensor("xs", [512 * SELF_CH, F_OUT], BF16,
                          kind="ExternalInput")
    iota_d = nc.dram_tensor("iota", [128, 2 * WIN], FP16,
                            kind="ExternalInput")
    rrep_d = nc.dram_tensor("rrep_in", [F_IN, NPAD], BF16,
                            kind="ExternalInput")
    rrep2_d = nc.dram_tensor("rrep_out", [F_IN, NPAD], BF16,
                             kind="ExternalInput")
    wbin_d = nc.dram_tensor("wb_in", [F_IN + 1, F_OUT], BF16,
                            kind="ExternalInput")
    wbout_d = nc.dram_tensor("wb_out", [F_IN + 1, F_OUT], BF16,
                             kind="ExternalInput")
    wb1_d = nc.dram_tensor("wb1", [F_IN + 1, F_HID], BF16,
                           kind="ExternalInput")
    w2p_d = nc.dram_tensor("w2p", [128, F_HID // 128 * F_OUT], BF16,
                           kind="ExternalInput")
    b2_d = nc.dram_tensor("b2", [1, F_OUT], BF16, kind="ExternalInput")

    dirs = []
    for name, wt, ntt in (("in", wt_in, nt_in), ("out", wt_out, nt_out)):
        dirs.append(dict(
            name=name, wtiles=wt, NT=ntt,
            gidx=nc.dram_tensor(f"gidx_{name}", [128, ntt * 8], I16,
                                kind="ExternalInput"),
            dlc=nc.dram_tensor(f"dlc_{name}", [128, ntt], FP16,
                               kind="ExternalInput"),
            yT=nc.dram_tensor(f"yT_{name}", [128, NPAD], F32,
                              kind="ExternalOutput"),
            wb=wbin_d if name == "in" else wbout_d,
            rrep=rrep_d if name == "in" else rrep2_d,
        ))
    yself_d = nc.dram_tensor("yT_self", [128, 512 * SELF_CH], F32,
                             kind="ExternalOutput")

    with tile.TileContext(nc) as tc:
        with tc.tile_pool(name="const", bufs=1) as cpool, \
             tc.tile_pool(name="meta", bufs=2) as mpool, \
             tc.tile_pool(name="idx", bufs=4) as ipool, \
             tc.tile_pool(name="feat", bufs=2) as fpool, \
             tc.tile_pool(name="featb", bufs=2) as fbpool, \
             tc.tile_pool(name="T", bufs=3) as tpool, \
             tc.tile_pool(name="ep", bufs=3) as epool, \
             tc.tile_pool(name="hacc", bufs=1) as hpool, \
             tc.tile_pool(name="rrep", bufs=1) as rpool, \
             tc.tile_pool(name="selfp", bufs=3) as spool, \
             tc.tile_pool(name="psAB", bufs=2, space="PSUM") as pspool, \
             tc.tile_pool(name="psY", bufs=2, space="PSUM") as pypool, \
             tc.tile_pool(name="ps2", bufs=2, space="PSUM") as ps2pool:

            iota_t = cpool.tile([128, 2 * WIN], FP16)
            nc.sync.dma_start(iota_t[:], iota_d[:])

            for d in dirs:
                NT = d["NT"]
                dlc_t = mpool.tile([128, NT], FP16, tag="dlc")
                nc.sync.dma_start(dlc_t[:], d["dlc"][:])
                rrep_t = rpool.tile([F_IN, NPAD], BF16, tag="rrep")
                nc.sync.dma_start(rrep_t[:], d["rrep"][:])
                wb_t = mpool.tile([F_IN + 1, F_OUT], BF16, tag="wb")
                nc.sync.dma_start(wb_t[:], d["wb"][:])
                hacc = hpool.tile([F_IN, NPAD], F32, tag="hacc")

                # chunk boundaries: consecutive tiles within one pass
                chunks = {}   # gt -> (chunk tiles, chunk first gt)
                gt = 0
                for b in range(2):
                    p0 = gt
                    ntp = sum(n for bb, _, n in d["wtiles"] if bb == b)
                    while gt < p0 + ntp:
                        n = min(CH_TILES, p0 + ntp - gt)
                        chunks[gt] = n
                        gt += n

                gt = 0
                feat_t = None
                featb_t = None
                ch0 = 0
                for (b, wi, ntw) in d["wtiles"]:
                    for k in range(ntw):
                        if gt in chunks:
                            ncht = chunks[gt]
                            ch0 = gt
                            gi = ipool.tile([128, ncht * 8], I16, tag="gi")
                            nc.sync.dma_start(
                                gi[:], d["gidx"][:, gt * 8:(gt + ncht) * 8])
                            feat_t = fpool.tile([128, CH_TILES, 128], F32,
                                                tag="feat")
                            src = xp[0:PBLK, :] if b == 0 else \
                                xp[PBLK:50048, :]
                            nidx = ncht * 128
                            nc.gpsimd.dma_gather(
                                feat_t[:, :ncht, :], src, gi[:], nidx, nidx,
                                128, single_packet=SP)
                            featb_t = fbpool.tile([128, CH_TILES, 128], BF16,
                                                  tag="featb")
                            nc.scalar.activation(
                                featb_t[:, :ncht, :].rearrange(
                                    "p n f -> p (n f)"),
                                feat_t[:, :ncht, :].rearrange(
                                    "p n f -> p (n f)"),
                                AF.Copy)
                        o = gt - ch0
                        if gt % TB == 0:
                            nb = min(TB, NT - gt)
                            Tb = tpool.tile([128, TB, 2 * WIN], BF16,
                                            tag="T")
                            i_b = iota_t[:].rearrange(
                                "p (one f) -> p one f", one=1).broadcast_to(
                                (128, nb, 2 * WIN))
                            d_b = dlc_t[:, gt:gt + nb].rearrange(
                                "p (k one) -> p k one", one=1).broadcast_to(
                                (128, nb, 2 * WIN))
                            nc.vector.tensor_tensor(Tb[:, :nb, :], i_b, d_b,
                                                    ALU.is_equal)
                            tb0 = gt
                        j = gt - tb0
                        if k == 0:
                            psAB = pspool.tile([F_IN, WIN], F32, tag="psAB")
                        nc.tensor.matmul(psAB[:],
                                         featb_t[:, o, 0:F_IN],
                                         Tb[:, j, 0:WIN],
                                         start=(k == 0), stop=False)
                        nc.tensor.matmul(psAB[:],
                                         featb_t[:, o, F_IN:128],
                                         Tb[:, j, WIN:2 * WIN],
                                         start=False, stop=(k == ntw - 1))
                        gt += 1

                    # window epilogue
                    wsl = slice(wi * WIN, (wi + 1) * WIN)
                    if b == 0:
                        nc.scalar.activation(hacc[:, wsl], psAB[:], AF.Copy)
                    else:
                        tmp = epool.tile([F_IN, WIN], BF16, tag="tmp")
                        nc.vector.tensor_tensor(
                            tmp[:], psAB[:], hacc[:, wsl], ALU.add)
                        hT = epool.tile([F_IN + 1, WIN], BF16, tag="hT")
                        nc.vector.tensor_tensor(
                            hT[0:F_IN, :], tmp[:], rrep_t[:, wsl], ALU.mult)
                        nc.vector.memset(hT[F_IN:F_IN + 1, :], 1.0)
                        psY = pypool.tile([128, WIN], F32, tag="psY")
                        nc.tensor.matmul(psY[:], wb_t[:], hT[:],
                                         start=True, stop=True)
                        y = epool.tile([128, WIN], F32, tag="y")
                        _emit_elu(nc, epool, psY[:], y[:], WIN, F32)
                        nc.sync.dma_start(d["yT"][:, wsl], y[:])

            # ---- self branch ----
            wb1 = cpool.tile([F_IN + 1, F_HID], BF16)
            nc.sync.dma_start(wb1[:], wb1_d[:])
            w2p = cpool.tile([128, F_HID // 128 * F_OUT], BF16)
            nc.sync.dma_start(w2p[:], w2p_d[:])
            b2t = cpool.tile([1, F_OUT], BF16)
            nc.sync.dma_start(b2t[:], b2_d[:])
            ones_row = cpool.tile([1, 512], BF16)
            nc.vector.memset(ones_row[:], 1.0)
            nk = F_HID // 128
            for t in range(SELF_CH):
                xT = spool.tile([128, 512], BF16, tag="xT")
                nc.scalar.dma_start_transpose(
                    xT[:], xs_d[512 * t:512 * (t + 1), :])
                ps2 = ps2pool.tile([128, 512], F32, tag="ps2")
                for k in range(nk):
                    ps1 = pypool.tile([128, 512], F32, tag="ps1")
                    nc.tensor.matmul(ps1[:], wb1[:, 128 * k:128 * (k + 1)],
                                     xT[0:F_IN + 1, :], start=True, stop=True)
                    hk = spool.tile([128, 512], BF16, tag="hk")
                    _emit_elu(nc, spool, ps1[:], hk[:], 512, BF16)
                    nc.tensor.matmul(ps2[:], w2p[:, F_OUT * k:F_OUT * (k + 1)],
                                     hk[:], start=(k == 0), stop=False)
                nc.tensor.matmul(ps2[:], b2t[:], ones_row[:],
                                 start=False, stop=True)
                yt = spool.tile([128, 512], F32, tag="yts")
                _emit_elu(nc, spool, ps2[:], yt[:], 512, F32)
                nc.sync.dma_start(yself_d[:, 512 * t:512 * (t + 1)], yt[:])

    nc.compile()
    return nc


def run(inputs, trace=False):
    x = np.asarray(inputs["x"], np.float32)
    ei = np.asarray(inputs["edge_index"], np.int64)
    src, dst = ei[0], ei[1]

    deg_dst = np.bincount(dst, minlength=N).astype(np.float32)
    deg_src = np.bincount(src, minlength=N).astype(np.float32)

    gin, dlin, wt_in, nt_in = prep_direction(dst, src, deg_dst)
    gout, dlout, wt_out, nt_out = prep_direction(src, dst, deg_src)

    xdev = np.zeros((NPAIR_ROWS, F_IN), np.float32)
    xdev[:N] = x

    # bf16 padded x with ones marker col for the self branch
    xb = np.zeros((NPAIR_ROWS, 128), np.float32)
    xb[:N, :F_IN] = x
    xb[:, F_IN] = 1.0
    xb16 = xb.astype(ml_dtypes.bfloat16)

    def bf(a):
        return np.asarray(a, np.float32).astype(ml_dtypes.bfloat16)

    wbin = np.vstack([inputs["W_in"], np.asarray(inputs["b_in"])[None, :]])
    wbout = np.vstack([inputs["W_out"], np.asarray(inputs["b_out"])[None, :]])
    wb1 = np.vstack([inputs["W1"], np.asarray(inputs["b1"])[None, :]])
    W2 = np.asarray(inputs["W2"], np.float32)
    w2p = np.zeros((128, (F_HID // 128) * F_OUT), np.float32)
    for k in range(F_HID // 128):
        w2p[:, k * F_OUT:(k + 1) * F_OUT] = W2[k * 128:(k + 1) * 128, :]
    b2 = np.asarray(inputs["b2"], np.float32)[None, :]

    iota = np.tile(np.arange(2 * WIN, dtype=np.float32)[None, :], (128, 1))
    iota16 = iota.astype(np.float16)

    def rrep_of(deg):
        r = np.ones((NCORES, NPAD), np.float32)
        for c in range(NCORES):
            dslice = deg[c * NPC:(c + 1) * NPC]
            r[c, :NPC] = 1.0 / np.maximum(dslice, 1.0)
        rr = np.repeat(r[:, None, :], F_IN, axis=1)   # [C, 64, NPAD]
        return rr.astype(ml_dtypes.bfloat16)

    rrep_in = rrep_of(deg_dst)
    rrep_out = rrep_of(deg_src)

    nc = build_nc(wt_in, nt_in, wt_out, nt_out)

    in_maps = []
    for c in range(NCORES):
        r0 = c * NPC
        xs = np.zeros((512 * SELF_CH, 128), ml_dtypes.bfloat16)
        take = min(512 * SELF_CH, NPAIR_ROWS - r0)
        xs[:take] = xb16[r0:r0 + take]
        in_maps.append({
            "x": xdev, "xs": xs, "iota": iota16,
            "wb_in": bf(wbin), "wb_out": bf(wbout), "wb1": bf(wb1),
            "w2p": bf(w2p), "b2": bf(b2),
            "gidx_in": gin[c], "dlc_in": dlin[c],
            "gidx_out": gout[c], "dlc_out": dlout[c],
            "rrep_in": rrep_in[c], "rrep_out": rrep_out[c],
        })

    kw = {}
    if trace:
        kw = dict(trace=True, trace_cores=[0])
    res = run_bass_kernel_spmd(nc, in_maps, core_ids=list(range(NCORES)),
                               **kw)

    def gather_out(name):
        return np.concatenate(
            [res.results[c][name][:, :NPC].T for c in range(NCORES)], 0)

    x_in = gather_out("yT_in")
    x_out = gather_out("yT_out")
    x_self = gather_out("yT_self")
    return (x_in, x_out, x_self), res


def kernel(**inputs):
    (x_in, x_out, x_self), _ = run(inputs, trace=False)
    return x_in, x_out, x_self


# revision 14
# speedup vs baseline: 1.7084x; 1.0880x over previous
"""DirSageConv Trainium2 kernel (8 NeuronCores, SPMD) — one-hot matmul design.

Strategy: nodes sharded across 8 cores; edges partitioned by destination
(resp. source) so the segment-mean lands in core-local PSUM windows; the
small linears are replicated.

Per direction (in: aggregate x[src] at dst; out: aggregate x[dst] at src):
  - Edges are grouped on the host by (core, src-pair block, dst-window) and
    padded to 128-edge tiles.  dst-window = 128 consecutive core-local keys.
  - dma_gather fetches the PAIR row (512B, two consecutive x rows, f32) of
    each edge's source node into SBUF in edge-slot order.  Pair indices fit
    int16 in two blocks (p < 32768, p >= 32768) -> two passes.
  - Scalar engine casts gathered chunks to bf16.
  - For each 128-edge tile, DVE builds a [128, 256] one-hot T via
    tensor_scalar(is_equal, dlc) * rec where dlc = dstlocal + 128*h
    (h = which row of the pair is the real source) and rec = 1/deg(dst).
    One matmul per tile accumulates  psum[pf, d0:128 | d1:128] += featb^T T.
  - Window epilogue: mean^T = psum[0:64, 0:128] + psum[64:128, 128:256]
    (pass A stashes this partial in SBUF; pass B adds it back), append a
    ones row, matmul with [W|b], exact ELU, DMA out (output transposed;
    host transposes back).
  - Self branch: two matmuls with ELU via xbar-transposed loads of x.
"""
import sys

sys.path.insert(0, "/opt/trn_rl_repo")

import ml_dtypes
import numpy as np

import concourse.bacc as bacc
import concourse.mybir as mybir
from concourse import tile
from concourse.bass_utils import run_bass_kernel_spmd

F32 = mybir.dt.float32
BF16 = mybir.dt.bfloat16
I16 = mybir.dt.int16
FP16 = mybir.dt.float16

AF = mybir.ActivationFunctionType
ALU = mybir.AluOpType

N = 100000
F_IN = 64
F_OUT = 128
F_HID = 512
NCORES = 8
NPC = N // NCORES              # 12500 nodes per core
WIN = 256                      # dsts per window
NWIN = -(-NPC // WIN)          # 49 windows of 256 dsts
NPAD = NWIN * WIN              # 12544
NPAIR_ROWS = 100096            # x padded rows (pairs: 50048)
PBLK = 32768                   # pair-index block size (int16 reach)
PAD_DLC = 600.0                # one-hot miss marker (> 511)
CH_TILES = 32                  # tiles per dma_gather chunk
SELF_CH = 25                   # 512-row self-branch chunks (12800 rows)
SP = False                     # single_packet=True wedges the device; keep False
TB = 4                         # tiles per one-hot build instruction


def _wrap_idx16(slots):
    """[C, TOT] int16 -> [C, 128, TOT//16] wrapped+replicated idx layout."""
    C, TOT = slots.shape
    a = slots.reshape(C, TOT // 16, 16).transpose(0, 2, 1)
    return np.tile(a, (1, 8, 1)).copy()


def prep_direction(key, val, deg):
    """Host edge partitioning for one direction.

    key: aggregation node (dst for 'in'), val: gathered node (src for 'in').
    Returns (gidx [C,128,cols], dlc [C,128,NT],
             wtiles list[(blk, w, ntiles)] shared across cores, NT).
    """
    E = key.shape[0]
    core = np.minimum(key // NPC, NCORES - 1)
    kl = key - core * NPC
    w = kl // WIN
    d = kl % WIN
    p = val >> 1
    h = val & 1
    blk = (p >= PBLK).astype(np.int64)

    # group id: (core, blk, w)
    gid = (core * 2 + blk) * NWIN + w
    NG = NCORES * 2 * NWIN
    cnt = np.bincount(gid, minlength=NG).reshape(NCORES, 2 * NWIN)
    tiles_bw = -(-cnt // 128)            # [C, 2*NWIN]
    tiles_shared = np.maximum(tiles_bw.max(axis=0), 1)   # [2*NWIN]
    NT = int(tiles_shared.sum())
    base = np.concatenate([[0], np.cumsum(tiles_shared)])[:-1]  # tile base

    order = np.argsort(gid, kind="stable")
    gs = gid[order]
    m = np.empty(E, np.bool_)
    m[0] = True
    np.not_equal(gs[1:], gs[:-1], out=m[1:])
    starts = np.flatnonzero(m)
    sid = np.cumsum(m) - 1
    within = np.arange(E) - starts[sid]
    pos = np.empty(E, np.int64)
    pos[order] = within
    bw = gid % (2 * NWIN)
    slot = base[bw] * 128 + pos          # slot within the core's stream

    TOT = NT * 128
    gidx = np.ones((NCORES, TOT), np.int16)       # pad: pair 1 (valid row)
    dlc_a = np.full((NCORES, TOT), PAD_DLC, np.float32)

    p16 = (p - blk * PBLK).astype(np.int16)
    gidx[core, slot] = p16
    dlc_a[core, slot] = (d + WIN * h).astype(np.float32)

    dlc = dlc_a.reshape(NCORES, NT, 128).transpose(0, 2, 1)
    dlc = np.ascontiguousarray(dlc).astype(np.float16)

    wtiles = []
    for b in range(2):
        for wi in range(NWIN):
            wtiles.append((b, wi, int(tiles_shared[b * NWIN + wi])))
    return _wrap_idx16(gidx), dlc, wtiles, NT


def _emit_elu(nc, pool, psum_ap, out_tile, n, out_dtype):
    """out = elu(psum) = min(exp(z)-1, relu(z))."""
    e = pool.tile([128, n], out_dtype, tag="elu_e")
    nc.scalar.activation(e[:, :n], psum_ap, AF.Exp)
    r = pool.tile([128, n], out_dtype, tag="elu_r")
    nc.vector.tensor_scalar_max(r[:, :n], psum_ap, 0.0)
    nc.vector.scalar_tensor_tensor(out_tile, e[:, :n], 1.0, r[:, :n],
                                   ALU.subtract, ALU.min)


def build_nc(wt_in, nt_in, wt_out, nt_out):
    nc = bacc.Bacc("TRN2", target_bir_lowering=False, debug=False,
                   enable_asserts=True, dynamic_dma_scratch_size=32768)

    x_d = nc.dram_tensor("x", [NPAIR_ROWS, F_IN], F32, kind="ExternalInput")
    xp = x_d.rearrange("(p two) f -> p (two f)", two=2)   # [50048, 128]
    xs_d = nc.dram_tensor("xs", [512 * SELF_CH, F_OUT], BF16,
                          kind="ExternalInput")
    iota_d = nc.dram_tensor("iota", [128, 2 * WIN], FP16,
                            kind="ExternalInput")
    rrep_d = nc.dram_tensor("rrep_in", [F_IN, NPAD], BF16,
                            kind="ExternalInput")
    rrep2_d = nc.dram_tensor("rrep_out", [F_IN, NPAD], BF16,
                             kind="ExternalInput")
    wbin_d = nc.dram_tensor("wb_in", [F_IN + 1, F_OUT], BF16,
                            kind="ExternalInput")
    wbout_d = nc.dram_tensor("wb_out", [F_IN + 1, F_OUT], BF16,
                             kind="ExternalInput")
    wb1_d = nc.dram_tensor("wb1", [F_IN + 1, F_HID], BF16,
                           kind="ExternalInput")
    w2p_d = nc.dram_tensor("w2p", [128, F_HID // 128 * F_OUT], BF16,
                           kind="ExternalInput")
    b2_d = nc.dram_tensor("b2", [1, F_OUT], BF16, kind="ExternalInput")

    dirs = []
    for name, wt, ntt in (("in", wt_in, nt_in), ("out", wt_out, nt_out)):
        dirs.append(dict(
            name=name, wtiles=wt, NT=ntt,
            gidx=nc.dram_tensor(f"gidx_{name}", [128, ntt * 8], I16,
                                kind="ExternalInput"),
            dlc=nc.dram_tensor(f"dlc_{name}", [128, ntt], FP16,
                               kind="ExternalInput"),
            yT=nc.dram_tensor(f"yT_{name}", [128, NPAD], F32,
                              kind="ExternalOutput"),
            wb=wbin_d if name == "in" else wbout_d,
            rrep=rrep_d if name == "in" else rrep2_d,
        ))
    yself_d = nc.dram_tensor("yT_self", [128, 512 * SELF_CH], F32,
                             kind="ExternalOutput")

    with tile.TileContext(nc) as tc:
        with tc.tile_pool(name="const", bufs=1) as cpool, \
             tc.tile_pool(name="meta", bufs=2) as mpool, \
             tc.tile_pool(name="idx", bufs=4) as ipool, \
             tc.tile_pool(name="feat", bufs=2) as fpool, \
             tc.tile_pool(name="featb", bufs=2) as fbpool, \
             tc.tile_pool(name="T", bufs=3) as tpool, \
             tc.tile_pool(name="ep", bufs=3) as epool, \
             tc.tile_pool(name="hacc", bufs=1) as hpool, \
             tc.tile_pool(name="rrep", bufs=1) as rpool, \
             tc.tile_pool(name="selfp", bufs=3) as spool, \
             tc.tile_pool(name="psAB", bufs=2, space="PSUM") as pspool, \
             tc.tile_pool(name="psY", bufs=2, space="PSUM") as pypool, \
             tc.tile_pool(name="ps2", bufs=2, space="PSUM") as ps2pool:

            iota_t = cpool.tile([128, 2 * WIN], FP16)
            nc.scalar.dma_start(iota_t[:], iota_d[:])

            # ---- self branch ----
            wb1 = cpool.tile([F_IN + 1, F_HID], BF16)
            nc.sync.dma_start(wb1[:], wb1_d[:])
            w2p = cpool.tile([128, F_HID // 128 * F_OUT], BF16)
            nc.sync.dma_start(w2p[:], w2p_d[:])
            b2t = cpool.tile([1, F_OUT], BF16)
            nc.sync.dma_start(b2t[:], b2_d[:])
            ones_row = cpool.tile([1, 512], BF16)
            nc.vector.memset(ones_row[:], 1.0)
            nk = F_HID // 128
            for t in range(SELF_CH):
                xT = spool.tile([128, 512], BF16, tag="xT")
                nc.scalar.dma_start_transpose(
                    xT[:], xs_d[512 * t:512 * (t + 1), :])
                ps2 = ps2pool.tile([128, 512], F32, tag="ps2")
                for k in range(nk):
                    ps1 = pypool.tile([128, 512], F32, tag="ps1")
                    nc.tensor.matmul(ps1[:], wb1[:, 128 * k:128 * (k + 1)],
                                     xT[0:F_IN + 1, :], start=True, stop=True)
                    hk = spool.tile([128, 512], BF16, tag="hk")
                    _emit_elu(nc, spool, ps1[:], hk[:], 512, BF16)
                    nc.tensor.matmul(ps2[:], w2p[:, F_OUT * k:F_OUT * (k + 1)],
                                     hk[:], start=(k == 0), stop=False)
                nc.tensor.matmul(ps2[:], b2t[:], ones_row[:],
                                 start=False, stop=True)
                yt = spool.tile([128, 512], F32, tag="yts")
                _emit_elu(nc, spool, ps2[:], yt[:], 512, F32)
                nc.sync.dma_start(yself_d[:, 512 * t:512 * (t + 1)], yt[:])


            for d in dirs:
                NT = d["NT"]
                dlc_t = mpool.tile([128, NT], FP16, tag="dlc")
                nc.scalar.dma_start(dlc_t[:], d["dlc"][:])
                rrep_t = rpool.tile([F_IN, NPAD], BF16, tag="rrep")
                nc.scalar.dma_start(rrep_t[:], d["rrep"][:])
                wb_t = mpool.tile([F_IN + 1, F_OUT], BF16, tag="wb")
                nc.scalar.dma_start(wb_t[:], d["wb"][:])
                hacc = hpool.tile([F_IN, NPAD], F32, tag="hacc")

                # chunk boundaries: consecutive tiles within one pass
                chunks = {}   # gt -> (chunk tiles, chunk first gt)
                gt = 0
                for b in range(2):
                    p0 = gt
                    ntp = sum(n for bb, _, n in d["wtiles"] if bb == b)
                    while gt < p0 + ntp:
                        n = min(CH_TILES, p0 + ntp - gt)
                        chunks[gt] = n
                        gt += n

                gt = 0
                feat_t = None
                featb_t = None
                ch0 = 0
                for (b, wi, ntw) in d["wtiles"]:
                    for k in range(ntw):
                        if gt in chunks:
                            ncht = chunks[gt]
                            ch0 = gt
                            gi = ipool.tile([128, ncht * 8], I16, tag="gi")
                            nc.sync.dma_start(
                                gi[:], d["gidx"][:, gt * 8:(gt + ncht) * 8])
                            feat_t = fpool.tile([128, CH_TILES, 128], F32,
                                                tag="feat")
                            src = xp[0:PBLK, :] if b == 0 else \
                                xp[PBLK:50048, :]
                            nidx = ncht * 128
                            nc.gpsimd.dma_gather(
                                feat_t[:, :ncht, :], src, gi[:], nidx, nidx,
                                128, single_packet=SP)
                            featb_t = fbpool.tile([128, CH_TILES, 128], BF16,
                                                  tag="featb")
                            nc.scalar.activation(
                                featb_t[:, :ncht, :].rearrange(
                                    "p n f -> p (n f)"),
                                feat_t[:, :ncht, :].rearrange(
                                    "p n f -> p (n f)"),
                                AF.Copy)
                        o = gt - ch0
                        if gt % TB == 0:
                            nb = min(TB, NT - gt)
                            Tb = tpool.tile([128, TB, 2 * WIN], BF16,
                                            tag="T")
                            i_b = iota_t[:].rearrange(
                                "p (one f) -> p one f", one=1).broadcast_to(
                                (128, nb, 2 * WIN))
                            d_b = dlc_t[:, gt:gt + nb].rearrange(
                                "p (k one) -> p k one", one=1).broadcast_to(
                                (128, nb, 2 * WIN))
                            nc.vector.tensor_tensor(Tb[:, :nb, :], i_b, d_b,
                                                    ALU.is_equal)
                            tb0 = gt
                        j = gt - tb0
                        if k == 0:
                            psAB = pspool.tile([F_IN, WIN], F32, tag="psAB")
                        nc.tensor.matmul(psAB[:],
                                         featb_t[:, o, 0:F_IN],
                                         Tb[:, j, 0:WIN],
                                         start=(k == 0), stop=False)
                        nc.tensor.matmul(psAB[:],
                                         featb_t[:, o, F_IN:128],
                                         Tb[:, j, WIN:2 * WIN],
                                         start=False, stop=(k == ntw - 1))
                        gt += 1

                    # window epilogue
                    wsl = slice(wi * WIN, (wi + 1) * WIN)
                    if b == 0:
                        nc.scalar.activation(hacc[:, wsl], psAB[:], AF.Copy)
                    else:
                        tmp = epool.tile([F_IN, WIN], BF16, tag="tmp")
                        nc.vector.tensor_tensor(
                            tmp[:], psAB[:], hacc[:, wsl], ALU.add)
                        hT = epool.tile([F_IN + 1, WIN], BF16, tag="hT")
                        nc.vector.tensor_tensor(
                            hT[0:F_IN, :], tmp[:], rrep_t[:, wsl], ALU.mult)
                        nc.vector.memset(hT[F_IN:F_IN + 1, :], 1.0)
                        psY = pypool.tile([128, WIN], F32, tag="psY")
                        nc.tensor.matmul(psY[:], wb_t[:], hT[:],
                                         start=True, stop=True)
                        y = epool.tile([128, WIN], F32, tag="y")
                        _emit_elu(nc, epool, psY[:], y[:], WIN, F32)
                        nc.sync.dma_start(d["yT"][:, wsl], y[:])

    nc.compile()
    return nc


def run(inputs, trace=False):
    x = np.asarray(inputs["x"], np.float32)
    ei = np.asarray(inputs["edge_index"], np.int64)
    src, dst = ei[0], ei[1]

    deg_dst = np.bincount(dst, minlength=N).astype(np.float32)
    deg_src = np.bincount(src, minlength=N).astype(np.float32)

    gin, dlin, wt_in, nt_in = prep_direction(dst, src, deg_dst)
    gout, dlout, wt_out, nt_out = prep_direction(src, dst, deg_src)

    xdev = np.zeros((NPAIR_ROWS, F_IN), np.float32)
    xdev[:N] = x

    # bf16 padded x with ones marker col for the self branch
    xb = np.zeros((NPAIR_ROWS, 128), np.float32)
    xb[:N, :F_IN] = x
    xb[:, F_IN] = 1.0
    xb16 = xb.astype(ml_dtypes.bfloat16)

    def bf(a):
        return np.asarray(a, np.float32).astype(ml_dtypes.bfloat16)

    wbin = np.vstack([inputs["W_in"], np.asarray(inputs["b_in"])[None, :]])
    wbout = np.vstack([inputs["W_out"], np.asarray(inputs["b_out"])[None, :]])
    wb1 = np.vstack([inputs["W1"], np.asarray(inputs["b1"])[None, :]])
    W2 = np.asarray(inputs["W2"], np.float32)
    w2p = np.zeros((128, (F_HID // 128) * F_OUT), np.float32)
    for k in range(F_HID // 128):
        w2p[:, k * F_OUT:(k + 1) * F_OUT] = W2[k * 128:(k + 1) * 128, :]
    b2 = np.asarray(inputs["b2"], np.float32)[None, :]

    iota = np.tile(np.arange(2 * WIN, dtype=np.float32)[None, :], (128, 1))
    iota16 = iota.astype(np.float16)

    def rrep_of(deg):
        r = np.ones((NCORES, NPAD), np.float32)
        for c in range(NCORES):
            dslice = deg[c * NPC:(c + 1) * NPC]
            r[c, :NPC] = 1.0 / np.maximum(dslice, 1.0)
        rr = np.repeat(r[:, None, :], F_IN, axis=1)   # [C, 64, NPAD]
        return rr.astype(ml_dtypes.bfloat16)

    rrep_in = rrep_of(deg_dst)
    rrep_out = rrep_of(deg_src)

    nc = build_nc(wt_in, nt_in, wt_out, nt_out)

    in_maps = []
    for c in range(NCORES):
        r0 = c * NPC
        xs = np.zeros((512 * SELF_CH, 128), ml_dtypes.bfloat16)
        take = min(512 * SELF_CH, NPAIR_ROWS - r0)
        xs[:take] = xb16[r0:r0 + take]
        in_maps.append({
            "x": xdev, "xs": xs, "iota": iota16,
            "wb_in": bf(wbin), "wb_out": bf(wbout), "wb1": bf(wb1),
            "w2p": bf(w2p), "b2": bf(b2),
            "gidx_in": gin[c], "dlc_in": dlin[c],
            "gidx_out": gout[c], "dlc_out": dlout[c],
            "rrep_in": rrep_in[c], "rrep_out": rrep_out[c],
        })

    kw = {}
    if trace:
        kw = dict(trace=True, trace_cores=[0])
    res = run_bass_kernel_spmd(nc, in_maps, core_ids=list(range(NCORES)),
                               **kw)

    def gather_out(name):
        return np.concatenate(
            [res.results[c][name][:, :NPC].T for c in range(NCORES)], 0)

    x_in = gather_out("yT_in")
    x_out = gather_out("yT_out")
    x_self = gather_out("yT_self")
    return (x_in, x_out, x_self), res


def kernel(**inputs):
    (x_in, x_out, x_self), _ = run(inputs, trace=False)
    return x_in, x_out, x_self
